# revision 7
# baseline (speedup 1.0000x reference)
"""Multi-head causal attention (B=2, S=2048, E=1024, H=16, D=64) on 8 trn2 cores.

Sharding (Megatron-style, per hint): data-parallel over batch (2) x
tensor-parallel over heads (4 groups of 4 heads / 256 features).
Core c: batch c//4, head-group c%4.

Per-core device program (SPMD, identical on all cores):
  1. PE-transpose x -> xT (contraction dim on partitions)
  2. qT/kT projections in [n, s] layout; v in natural [s, n] layout
  3. causal attention in transposed-score layout:
       sT[j,i] = kT_h . qT_h (K=64 matmul), p = exp(s/8) on ScalarE,
       causal mask via multiplicative 0/1 tiles on diagonal blocks,
       ctxT[d,i] accumulated with v-augmented-with-ones stationary ->
       row 64 of psum = softmax denominator; normalize via
       reciprocal + partition_broadcast + multiply
  4. AllGather ctxT across the 4-core batch group -> full [1024, 2048]
  5. out[:, g*256:(g+1)*256] = ctxT_full.T @ Wo[:, slice] + bo[slice]
Host only slices inputs and concatenates the 8 disjoint output slices.
"""

import contextlib

import numpy as np

import concourse.mybir as mybir
import concourse.tile as tile
from concourse import bacc
from concourse.bass_utils import run_bass_kernel_spmd

F32 = mybir.dt.float32
F32R = mybir.dt.float32r

B, S, E, H, D = 2, 2048, 1024, 16, 64
N_CORES = 8
TP = 4                 # tensor-parallel degree (head groups per batch)
NSL = E // TP          # 256 features per core
HLOC = H // TP         # 4 heads per core
KT = E // 128          # 8 contraction tiles
IT = S // 128          # 16 sequence tiles
ICH = S // 512         # 4 sequence chunks of 512
SCALE = 1.0 / np.sqrt(D)

REPLICA_GROUPS = [[0, 1, 2, 3], [4, 5, 6, 7]]

_cache: dict = {}


def _emit(nc, tc, prm):
    x, wq, bq, wk, bk, wv, bv, wo, bo, ident, masks, out = prm

    with contextlib.ExitStack() as stack:
        const = stack.enter_context(tc.tile_pool(name="const", bufs=1))
        wpool = stack.enter_context(tc.tile_pool(name="wpool", bufs=1))
        psum_t = stack.enter_context(tc.tile_pool(name="psum_t", bufs=2, space="PSUM"))
        psum_mm = stack.enter_context(tc.tile_pool(name="psum_mm", bufs=2, space="PSUM"))
        psum_s = stack.enter_context(tc.tile_pool(name="psum_s", bufs=2, space="PSUM"))
        psum_c = stack.enter_context(tc.tile_pool(name="psum_c", bufs=2, space="PSUM"))
        osb_p = stack.enter_context(tc.tile_pool(name="osb", bufs=2))
        dram = stack.enter_context(tc.tile_pool(name="dram", bufs=1, space="DRAM"))

        # ---- constants ----
        ident_sb = const.tile([128, 128], F32)
        nc.sync.dma_start(out=ident_sb[:], in_=ident[:])
        mask_sb = const.tile([128, HLOC, 512], F32)
        nc.sync.dma_start(out=mask_sb[:], in_=masks[:])

        # ---- weights (wo/bo loaded in stage E) ----
        wq_sb = wpool.tile([128, KT, NSL], F32R)
        wk_sb = wpool.tile([128, KT, NSL], F32R)
        wv_sb = wpool.tile([128, KT, NSL], F32R)
        for w_sb, w_dr in ((wq_sb, wq), (wk_sb, wk), (wv_sb, wv)):
            nc.sync.dma_start(out=w_sb[:], in_=w_dr.rearrange("(t p) n -> p t n", p=128).bitcast(F32R))
        bq_sb = wpool.tile([128, 2], F32)
        bk_sb = wpool.tile([128, 2], F32)
        for b_sb, b_dr in ((bq_sb, bq), (bk_sb, bk)):
            nc.sync.dma_start(out=b_sb[:], in_=b_dr.rearrange("(t p) -> p t", p=128))
        bv_row = wpool.tile([1, NSL], F32)
        nc.sync.dma_start(out=bv_row[:], in_=bv[None, :])
        bvb = wpool.tile([128, NSL], F32)
        nc.gpsimd.partition_broadcast(out_ap=bvb[:], in_ap=bv_row[:])

        with contextlib.ExitStack() as stage_bc:
            qkv_p = stage_bc.enter_context(tc.tile_pool(name="qkv", bufs=1))
            pwork = stage_bc.enter_context(tc.tile_pool(name="pwork", bufs=3))
            norm_p = stage_bc.enter_context(tc.tile_pool(name="norm", bufs=2))
            ctxt_p = stage_bc.enter_context(tc.tile_pool(name="ctxt", bufs=1))

            qt_sb = qkv_p.tile([128, 2, S], F32R)
            kt_sb = qkv_p.tile([128, 2, S], F32R)
            v_sb = qkv_p.tile([128, IT, HLOC, D + 1], F32R)
            ones_col = qkv_p.tile([128, IT, HLOC, 1], F32)
            nc.vector.memset(ones_col[:], 1.0)
            nc.vector.tensor_copy(v_sb[:, :, :, D:D + 1], ones_col[:])

            with contextlib.ExitStack() as stage_a:
                xrow_p = stage_a.enter_context(tc.tile_pool(name="xrow", bufs=3))
                xt_p = stage_a.enter_context(tc.tile_pool(name="xt", bufs=1))

                # ---- stage A: x -> xT ----
                xt_sb = xt_p.tile([128, KT, S], F32R)
                for it in range(IT):
                    xr = xrow_p.tile([128, E], F32, tag="xr")
                    nc.sync.dma_start(out=xr[:], in_=x[it * 128:(it + 1) * 128, :])
                    for kt in range(KT):
                        pt = psum_t.tile([128, 128], F32, tag="pt")
                        nc.tensor.transpose(
                            pt[:], xr[:, kt * 128:(kt + 1) * 128], ident_sb[:])
                        nc.scalar.copy(
                            out=xt_sb[:, kt, it * 128:(it + 1) * 128], in_=pt[:])

                # ---- stage B: qT/kT [n, s], v [s, n] ----
                for (w_sb, b_sb, o_sb) in ((wq_sb, bq_sb, qt_sb),
                                           (wk_sb, bk_sb, kt_sb)):
                    for nt in range(2):
                        for ic in range(ICH):
                            pm = psum_mm.tile([128, 512], F32, tag="pqk")
                            for kt in range(KT):
                                nc.tensor.matmul(
                                    pm[:],
                                    w_sb[:, kt, nt * 128:(nt + 1) * 128],
                                    xt_sb[:, kt, ic * 512:(ic + 1) * 512],
                                    start=(kt == 0), stop=(kt == KT - 1),
                                )
                            nc.vector.tensor_scalar_add(
                                out=o_sb[:, nt, ic * 512:(ic + 1) * 512],
                                in0=pm[:], scalar1=b_sb[:, nt:nt + 1])

                for it in range(IT):
                    pv = psum_mm.tile([128, NSL], F32, tag="pqk")
                    for kt in range(KT):
                        nc.tensor.matmul(
                            pv[:],
                            xt_sb[:, kt, it * 128:(it + 1) * 128],
                            wv_sb[:, kt, :],
                            start=(kt == 0), stop=(kt == KT - 1),
                        )
                    nc.vector.tensor_add(
                        out=v_sb[:, it, :, 0:D],
                        in0=pv[:].rearrange("p (h d) -> p h d", d=D),
                        in1=bvb[:].rearrange("p (h d) -> p h d", d=D))

            # ---- stage C: attention ----
            ctxt_sb = ctxt_p.tile([128, 2, S], F32R)
            for h in range(HLOC):
                nt, base = divmod(h, 2)
                base *= D
                qth = qt_sb[base:base + D, nt, :]
                kth = kt_sb[base:base + D, nt, :]
                for ic in range(ICH):
                    i0 = ic * 512
                    pc = psum_c.tile([D + 1, 512], F32, tag="pc")
                    njt = 4 * (ic + 1)
                    for jt in range(njt):
                        ps = psum_s.tile([128, 512], F32, tag="ps")
                        nc.tensor.matmul(
                            ps[:],
                            kth[:, jt * 128:(jt + 1) * 128],
                            qth[:, i0:i0 + 512],
                            start=True, stop=True,
                        )
                        pw = pwork.tile([128, 512], F32R, tag="pw")
                        nc.scalar.activation(
                            out=pw[:], in_=ps[:],
                            func=mybir.ActivationFunctionType.Exp,
                            scale=float(SCALE))
                        dt_ = jt - 4 * ic
                        if dt_ >= 0:
                            nc.vector.tensor_mul(pw[:], pw[:], mask_sb[:, dt_, :])
                        nc.tensor.matmul(
                            pc[:],
                            v_sb[:, jt, h, :],
                            pw[:],
                            start=(jt == 0), stop=(jt == njt - 1),
                        )
                    rl = norm_p.tile([1, 512], F32, tag="rl")
                    nc.vector.reciprocal(out=rl[:], in_=pc[D:D + 1, :])
                    rlb = norm_p.tile([D, 512], F32, tag="rlb")
                    nc.gpsimd.partition_broadcast(out_ap=rlb[:], in_ap=rl[:])
                    nc.vector.tensor_mul(
                        ctxt_sb[base:base + D, nt, i0:i0 + 512],
                        pc[0:D, :], rlb[:])

            # ---- stage D: allgather ctxT over the 4-core batch group ----
            cc_in = dram.tile([NSL, S], F32R)
            cc_out = dram.tile([TP, NSL, S], F32R)
            nc.sync.dma_start(
                out=cc_in.rearrange("(t p) i -> p t i", p=128), in_=ctxt_sb[:])
            nc.gpsimd.collective_compute(
                "AllGather", mybir.AluOpType.bypass,
                replica_groups=REPLICA_GROUPS,
                ins=[cc_in.opt()], outs=[cc_out.opt()],
            )

        # ---- stage E: out projection ----
        with contextlib.ExitStack() as stage_e:
            ctxf_p = stage_e.enter_context(tc.tile_pool(name="ctxf", bufs=1))
            wout_p = stage_e.enter_context(tc.tile_pool(name="wout", bufs=1))

            wo_sb = wout_p.tile([128, KT, NSL], F32R)
            nc.sync.dma_start(out=wo_sb[:], in_=wo.rearrange("(t p) n -> p t n", p=128).bitcast(F32R))
            bo_row = wout_p.tile([1, NSL], F32)
            nc.sync.dma_start(out=bo_row[:], in_=bo[None, :])
            bob = wout_p.tile([128, NSL], F32)
            nc.gpsimd.partition_broadcast(out_ap=bob[:], in_ap=bo_row[:])

            ctxf_sb = ctxf_p.tile([128, KT, S], F32R)
            nc.sync.dma_start(
                out=ctxf_sb[:],
                in_=cc_out.rearrange("g (t p) i -> p (g t) i", p=128))

            for it in range(IT):
                po = psum_mm.tile([128, NSL], F32, tag="pqk")
                for ct in range(KT):
                    nc.tensor.matmul(
                        po[:],
                        ctxf_sb[:, ct, it * 128:(it + 1) * 128],
                        wo_sb[:, ct, :],
                        start=(ct == 0), stop=(ct == KT - 1),
                    )
                ot = osb_p.tile([128, NSL], F32, tag="ot")
                nc.vector.tensor_add(out=ot[:], in0=po[:], in1=bob[:])
                nc.sync.dma_start(out=out[it * 128:(it + 1) * 128, :], in_=ot[:])


def _build():
    nc = bacc.Bacc("TRN2", target_bir_lowering=False, debug=False,
                   num_devices=N_CORES)
    x = nc.declare_dram_parameter("x", [S, E], F32, isOutput=False).ap()
    wq = nc.declare_dram_parameter("wq", [E, NSL], F32, isOutput=False).ap()
    bq = nc.declare_dram_parameter("bq", [NSL], F32, isOutput=False).ap()
    wk = nc.declare_dram_parameter("wk", [E, NSL], F32, isOutput=False).ap()
    bk = nc.declare_dram_parameter("bk", [NSL], F32, isOutput=False).ap()
    wv = nc.declare_dram_parameter("wv", [E, NSL], F32, isOutput=False).ap()
    bv = nc.declare_dram_parameter("bv", [NSL], F32, isOutput=False).ap()
    wo = nc.declare_dram_parameter("wo", [E, NSL], F32, isOutput=False).ap()
    bo = nc.declare_dram_parameter("bo", [NSL], F32, isOutput=False).ap()
    ident = nc.declare_dram_parameter("ident", [128, 128], F32, isOutput=False).ap()
    masks = nc.declare_dram_parameter("masks", [128, HLOC, 512], F32,
                                      isOutput=False).ap()
    out = nc.declare_dram_parameter("out", [S, NSL], F32, isOutput=True).ap()

    with tile.TileContext(nc) as tc:
        _emit(nc, tc, (x, wq, bq, wk, bk, wv, bv, wo, bo, ident, masks, out))
    nc.compile()
    return nc


def _consts():
    ident = np.eye(128, dtype=np.float32)
    jl = np.arange(128, dtype=np.int64)[:, None]
    il = np.arange(512, dtype=np.int64)[None, :]
    masks = np.stack(
        [(il >= jl + 128 * d).astype(np.float32) for d in range(HLOC)], axis=1)
    return ident, np.ascontiguousarray(masks)


def kernel(x, Wq, bq, Wk, bk, Wv, bv, Wo, bo, _trace=False, _trace_cores=None):
    if "nc" not in _cache:
        _cache["nc"] = _build()
    nc = _cache["nc"]
    ident, masks = _consts()
    x = np.asarray(x, dtype=np.float32)
    in_maps = []
    for c in range(N_CORES):
        bi, g = divmod(c, TP)
        sl = slice(g * NSL, (g + 1) * NSL)
        in_maps.append({
            "x": np.ascontiguousarray(x[bi]),
            "wq": np.ascontiguousarray(np.asarray(Wq)[:, sl]),
            "bq": np.ascontiguousarray(np.asarray(bq)[sl]),
            "wk": np.ascontiguousarray(np.asarray(Wk)[:, sl]),
            "bk": np.ascontiguousarray(np.asarray(bk)[sl]),
            "wv": np.ascontiguousarray(np.asarray(Wv)[:, sl]),
            "bv": np.ascontiguousarray(np.asarray(bv)[sl]),
            "wo": np.ascontiguousarray(np.asarray(Wo)[:, sl]),
            "bo": np.ascontiguousarray(np.asarray(bo)[sl]),
            "ident": ident,
            "masks": masks,
        })
    res = run_bass_kernel_spmd(
        nc, in_maps, list(range(N_CORES)),
        trace=_trace, trace_cores=_trace_cores)
    out = np.empty((B, S, E), np.float32)
    for c in range(N_CORES):
        bi, g = divmod(c, TP)
        out[bi, :, g * NSL:(g + 1) * NSL] = res.results[c]["out"]
    if _trace:
        _cache["last_result"] = res
    return out


# revision 11
# speedup vs baseline: 1.5928x; 1.5928x over previous
"""Multi-head causal attention (B=2, S=2048, E=1024, H=16, D=64) on 8 trn2 cores.

Sharding (Megatron-style, per hint): data-parallel over batch (2) x
tensor-parallel over heads (4 groups of 4 heads / 256 features).
Core c: batch c//4, head-group c%4.

Per-core device program (SPMD, identical on all cores), pipelined over 4
query chunks of 512:
  A(ic). PE-transpose x rows -> xT (bf16) for the chunk's 4 i-tiles
  B(ic). qT/kT projections in [n, s] layout; v in natural [s, n] layout (bf16)
  C(ic). causal attention in transposed-score layout:
       sT[j,i] = kT_h . qT_h (K=64 matmul), p = exp(s/8) on ScalarE (bf16),
       causal mask via multiplicative 0/1 tiles on diagonal blocks,
       ctxT[d,i] accumulated with v-augmented-with-ones stationary ->
       row 64 of psum = softmax denominator; normalize with
       copy + partition_broadcast + tensor_tensor divide
  D(ic). AllGather the normalized ctxT chunk across the 4-core batch group
  E(ic). out[:, g*256:(g+1)*256] = ctxT_full.T @ Wo[:, slice] + bo[slice]
       (E is emitted one chunk behind so the collective hides behind compute)
Host only slices inputs and concatenates the 8 disjoint output slices.
"""

import contextlib

import ml_dtypes
import numpy as np

import concourse.mybir as mybir
import concourse.tile as tile
from concourse import bacc
from concourse.bass_utils import run_bass_kernel_spmd

F32 = mybir.dt.float32
BF16 = mybir.dt.bfloat16

B, S, E, H, D = 2, 2048, 1024, 16, 64
N_CORES = 8
TP = 4                 # tensor-parallel degree (head groups per batch)
NSL = E // TP          # 256 features per core
HLOC = H // TP         # 4 heads per core
KT = E // 128          # 8 contraction tiles
IT = S // 128          # 16 sequence tiles
ICH = S // 512         # 4 sequence chunks of 512
SCALE = 1.0 / np.sqrt(D)

REPLICA_GROUPS = [[0, 1, 2, 3], [4, 5, 6, 7]]

_cache: dict = {}


def _emit(nc, tc, prm):
    x, wq, bq, wk, bk, wv, bv, wo, bo, ident, masks, out = prm

    with contextlib.ExitStack() as stack:
        ent = stack.enter_context
        const = ent(tc.tile_pool(name="const", bufs=1))
        wstage = ent(tc.tile_pool(name="wstage", bufs=2))
        wpool = ent(tc.tile_pool(name="wpool", bufs=1))
        xrow_p = ent(tc.tile_pool(name="xrow", bufs=3))
        xt_p = ent(tc.tile_pool(name="xt", bufs=1))
        qkv_p = ent(tc.tile_pool(name="qkv", bufs=1))
        psum_t = ent(tc.tile_pool(name="psum_t", bufs=2, space="PSUM"))
        psum_mm = ent(tc.tile_pool(name="psum_mm", bufs=2, space="PSUM"))
        psum_s = ent(tc.tile_pool(name="psum_s", bufs=2, space="PSUM"))
        psum_c = ent(tc.tile_pool(name="psum_c", bufs=2, space="PSUM"))
        pwork = ent(tc.tile_pool(name="pwork", bufs=4))
        norm_p = ent(tc.tile_pool(name="norm", bufs=2))
        ctxt_p = ent(tc.tile_pool(name="ctxt", bufs=1))
        ctxf_p = ent(tc.tile_pool(name="ctxf", bufs=2))
        osb_p = ent(tc.tile_pool(name="osb", bufs=2))
        dram = ent(tc.tile_pool(name="dram", bufs=1, space="DRAM"))

        # ---- constants ----
        ident_sb = const.tile([128, 128], F32)
        nc.sync.dma_start(out=ident_sb[:], in_=ident[:])
        mask_sb = const.tile([128, HLOC, 512], BF16)
        nc.sync.dma_start(out=mask_sb[:], in_=masks[:])

        # ---- weights: load f32, convert to bf16 on DVE ----
        wq_sb = wpool.tile([128, KT, NSL], BF16)
        wk_sb = wpool.tile([128, KT, NSL], BF16)
        wv_sb = wpool.tile([128, KT, NSL], BF16)
        wo_sb = wpool.tile([128, KT, NSL], BF16)
        for w_sb, w_dr in ((wq_sb, wq), (wk_sb, wk), (wv_sb, wv), (wo_sb, wo)):
            wst = wstage.tile([128, KT, NSL], F32, tag="wst")
            nc.sync.dma_start(out=wst[:], in_=w_dr.rearrange("(t p) n -> p t n", p=128))
            nc.vector.tensor_copy(w_sb[:], wst[:])
        bq_sb = wpool.tile([128, 2], F32)
        bk_sb = wpool.tile([128, 2], F32)
        for b_sb, b_dr in ((bq_sb, bq), (bk_sb, bk)):
            nc.sync.dma_start(out=b_sb[:], in_=b_dr.rearrange("(t p) -> p t", p=128))
        bv_row = wpool.tile([1, NSL], F32)
        nc.sync.dma_start(out=bv_row[:], in_=bv[None, :])
        bvb = wpool.tile([128, NSL], F32)
        nc.gpsimd.partition_broadcast(out_ap=bvb[:], in_ap=bv_row[:])
        bo_row = wpool.tile([1, NSL], F32)
        nc.sync.dma_start(out=bo_row[:], in_=bo[None, :])
        bob = wpool.tile([128, NSL], F32)
        nc.gpsimd.partition_broadcast(out_ap=bob[:], in_ap=bo_row[:])

        # ---- persistent activations ----
        xt_sb = xt_p.tile([128, KT, S], BF16)
        qt_sb = qkv_p.tile([128, 2, S], BF16)
        kt_sb = qkv_p.tile([128, 2, S], BF16)
        v_sb = qkv_p.tile([128, IT, HLOC, D + 1], BF16)
        ones_col = qkv_p.tile([128, IT, HLOC, 1], F32)
        nc.vector.memset(ones_col[:], 1.0)
        nc.vector.tensor_copy(v_sb[:, :, :, D:D + 1], ones_col[:])
        ctxt_sb = ctxt_p.tile([128, 2, S], BF16)

        # DRAM bounce buffers for the chunked allgather (distinct per chunk
        # so chunk ic+1's send never waits on chunk ic's collective)
        cc_in = [dram.tile([NSL, 512], BF16, name=f"cc_in{ic}") for ic in range(ICH)]
        cc_out = [dram.tile([TP, NSL, 512], BF16, name=f"cc_out{ic}")
                  for ic in range(ICH)]

        def stage_a(ic):
            for it in range(4 * ic, 4 * ic + 4):
                xr = xrow_p.tile([128, E], F32, tag="xr")
                nc.sync.dma_start(out=xr[:], in_=x[it * 128:(it + 1) * 128, :])
                for kt in range(KT):
                    pt = psum_t.tile([128, 128], F32, tag="pt")
                    nc.tensor.transpose(
                        pt[:], xr[:, kt * 128:(kt + 1) * 128], ident_sb[:])
                    nc.scalar.copy(
                        out=xt_sb[:, kt, it * 128:(it + 1) * 128], in_=pt[:])

        def stage_b(ic):
            for (w_sb, b_sb, o_sb) in ((wq_sb, bq_sb, qt_sb),
                                       (wk_sb, bk_sb, kt_sb)):
                for nt in range(2):
                    pm = psum_mm.tile([128, 512], F32, tag="pqk")
                    for kt in range(KT):
                        nc.tensor.matmul(
                            pm[:],
                            w_sb[:, kt, nt * 128:(nt + 1) * 128],
                            xt_sb[:, kt, ic * 512:(ic + 1) * 512],
                            start=(kt == 0), stop=(kt == KT - 1),
                        )
                    nc.vector.tensor_scalar_add(
                        out=o_sb[:, nt, ic * 512:(ic + 1) * 512],
                        in0=pm[:], scalar1=b_sb[:, nt:nt + 1])
            for it in range(4 * ic, 4 * ic + 4):
                pv = psum_mm.tile([128, NSL], F32, tag="pqk")
                for kt in range(KT):
                    nc.tensor.matmul(
                        pv[:],
                        xt_sb[:, kt, it * 128:(it + 1) * 128],
                        wv_sb[:, kt, :],
                        start=(kt == 0), stop=(kt == KT - 1),
                    )
                nc.vector.tensor_add(
                    out=v_sb[:, it, :, 0:D],
                    in0=pv[:].rearrange("p (h d) -> p h d", d=D),
                    in1=bvb[:].rearrange("p (h d) -> p h d", d=D))

        def stage_c(ic):
            i0 = ic * 512
            for h in range(HLOC):
                nt, base = divmod(h, 2)
                base *= D
                qth = qt_sb[base:base + D, nt, :]
                kth = kt_sb[base:base + D, nt, :]
                pc = psum_c.tile([D + 1, 512], F32, tag="pc")
                njt = 4 * (ic + 1)
                for jt in range(njt):
                    ps = psum_s.tile([128, 512], F32, tag="ps")
                    nc.tensor.matmul(
                        ps[:],
                        kth[:, jt * 128:(jt + 1) * 128],
                        qth[:, i0:i0 + 512],
                        start=True, stop=True,
                    )
                    pw = pwork.tile([128, 512], BF16, tag="pw")
                    nc.scalar.activation(
                        out=pw[:], in_=ps[:],
                        func=mybir.ActivationFunctionType.Exp,
                        scale=float(SCALE))
                    dt_ = jt - 4 * ic
                    if dt_ >= 0:
                        nc.vector.tensor_mul(pw[:], pw[:], mask_sb[:, dt_, :])
                    nc.tensor.matmul(
                        pc[:],
                        v_sb[:, jt, h, :],
                        pw[:],
                        start=(jt == 0), stop=(jt == njt - 1),
                    )
                lrow = norm_p.tile([1, 512], F32, tag="lrow")
                nc.vector.reciprocal(out=lrow[:], in_=pc[D:D + 1, :])
                lb = norm_p.tile([D, 512], F32, tag="lb")
                nc.gpsimd.partition_broadcast(out_ap=lb[:], in_ap=lrow[:])
                nc.vector.tensor_mul(
                    ctxt_sb[base:base + D, nt, i0:i0 + 512],
                    pc[0:D, :], lb[:])

        def stage_d(ic):
            i0 = ic * 512
            nc.sync.dma_start(
                out=cc_in[ic].rearrange("(t p) i -> p t i", p=128),
                in_=ctxt_sb[:, :, i0:i0 + 512])
            nc.gpsimd.collective_compute(
                "AllGather", mybir.AluOpType.bypass,
                replica_groups=REPLICA_GROUPS,
                ins=[cc_in[ic].opt()], outs=[cc_out[ic].opt()],
            )

        def stage_e(ic):
            ctxf_sb = ctxf_p.tile([128, KT, 512], BF16, tag="ctxf")
            nc.sync.dma_start(
                out=ctxf_sb[:],
                in_=cc_out[ic].rearrange("g (t p) i -> p (g t) i", p=128))
            for k, it in enumerate(range(4 * ic, 4 * ic + 4)):
                po = psum_mm.tile([128, NSL], F32, tag="pqk")
                for ct in range(KT):
                    nc.tensor.matmul(
                        po[:],
                        ctxf_sb[:, ct, k * 128:(k + 1) * 128],
                        wo_sb[:, ct, :],
                        start=(ct == 0), stop=(ct == KT - 1),
                    )
                ot = osb_p.tile([128, NSL], F32, tag="ot")
                nc.vector.tensor_add(out=ot[:], in0=po[:], in1=bob[:])
                nc.sync.dma_start(out=out[it * 128:(it + 1) * 128, :], in_=ot[:])

        # pipeline: E(ic) is emitted one chunk late so its PE work sits
        # behind the next chunk's compute while the allgather completes
        for ic in range(ICH):
            stage_a(ic)
            stage_b(ic)
            stage_c(ic)
            stage_d(ic)
            if ic > 0:
                stage_e(ic - 1)
        stage_e(ICH - 1)


def _build():
    nc = bacc.Bacc("TRN2", target_bir_lowering=False, debug=False,
                   num_devices=N_CORES)
    x = nc.declare_dram_parameter("x", [S, E], F32, isOutput=False).ap()
    wq = nc.declare_dram_parameter("wq", [E, NSL], F32, isOutput=False).ap()
    bq = nc.declare_dram_parameter("bq", [NSL], F32, isOutput=False).ap()
    wk = nc.declare_dram_parameter("wk", [E, NSL], F32, isOutput=False).ap()
    bk = nc.declare_dram_parameter("bk", [NSL], F32, isOutput=False).ap()
    wv = nc.declare_dram_parameter("wv", [E, NSL], F32, isOutput=False).ap()
    bv = nc.declare_dram_parameter("bv", [NSL], F32, isOutput=False).ap()
    wo = nc.declare_dram_parameter("wo", [E, NSL], F32, isOutput=False).ap()
    bo = nc.declare_dram_parameter("bo", [NSL], F32, isOutput=False).ap()
    ident = nc.declare_dram_parameter("ident", [128, 128], F32, isOutput=False).ap()
    masks = nc.declare_dram_parameter("masks", [128, HLOC, 512], BF16,
                                      isOutput=False).ap()
    out = nc.declare_dram_parameter("out", [S, NSL], F32, isOutput=True).ap()

    with tile.TileContext(nc) as tc:
        _emit(nc, tc, (x, wq, bq, wk, bk, wv, bv, wo, bo, ident, masks, out))
    nc.compile()
    return nc


def _consts():
    ident = np.eye(128, dtype=np.float32)
    jl = np.arange(128, dtype=np.int64)[:, None]
    il = np.arange(512, dtype=np.int64)[None, :]
    masks = np.stack(
        [(il >= jl + 128 * d).astype(ml_dtypes.bfloat16) for d in range(HLOC)],
        axis=1)
    return ident, np.ascontiguousarray(masks)


def kernel(x, Wq, bq, Wk, bk, Wv, bv, Wo, bo, _trace=False, _trace_cores=None):
    if "nc" not in _cache:
        _cache["nc"] = _build()
    nc = _cache["nc"]
    ident, masks = _consts()
    x = np.asarray(x, dtype=np.float32)
    in_maps = []
    for c in range(N_CORES):
        bi, g = divmod(c, TP)
        sl = slice(g * NSL, (g + 1) * NSL)
        in_maps.append({
            "x": np.ascontiguousarray(x[bi]),
            "wq": np.ascontiguousarray(np.asarray(Wq)[:, sl]),
            "bq": np.ascontiguousarray(np.asarray(bq)[sl]),
            "wk": np.ascontiguousarray(np.asarray(Wk)[:, sl]),
            "bk": np.ascontiguousarray(np.asarray(bk)[sl]),
            "wv": np.ascontiguousarray(np.asarray(Wv)[:, sl]),
            "bv": np.ascontiguousarray(np.asarray(bv)[sl]),
            "wo": np.ascontiguousarray(np.asarray(Wo)[:, sl]),
            "bo": np.ascontiguousarray(np.asarray(bo)[sl]),
            "ident": ident,
            "masks": masks,
        })
    res = run_bass_kernel_spmd(
        nc, in_maps, list(range(N_CORES)),
        trace=_trace, trace_cores=_trace_cores)
    out = np.empty((B, S, E), np.float32)
    for c in range(N_CORES):
        bi, g = divmod(c, TP)
        out[bi, :, g * NSL:(g + 1) * NSL] = res.results[c]["out"]
    if _trace:
        _cache["last_result"] = res
    return out
